# revision 3
# baseline (speedup 1.0000x reference)
"""Fused Conv3x3-InstanceNorm-ReLU x2 block for Trainium2 (fp16 path).

Data-parallel over 8 NeuronCores (one batch sample per core). Per-core:

  pass A: conv1 as row-pair matmuls (fp16, K=128 = 4 row-shifted Cin
          blocks, M=128 = 2 out rows x 64 Cout, N=320, fp32 PSUM).
          x is host-tiled into 20 contiguous [128, 8*324] fp16 groups
          (pads and edge rows pre-zeroed) so pass A needs exactly 20
          big DMAs. Per pair: ACT evacuates PSUM into a big fp16 SBUF
          arena; DVE bn_stats records (count, mean, M2) per partition.
  norm1:  h = relu(y1 - mu1) (DVE, fp16 4x mode); the rsqrt scale s1
          is folded into the conv2 weights per input channel.
  pass B: conv2 on normalized pairs (2 K-blocks of 64 ch, 6 matmuls).
          y2 lands fp16 IN PLACE over the arena slot its eA input just
          retired -- zero spill, zero extra SBUF. Row 0/319 singles run
          after the pair loop so PE enters pass B sooner.
  stats:  per-half bn_aggr (equal-count records per half) + count-
          weighted combine across the two row-parity halves.
  pass C: out = relu((y2 - mu2) * s2), alternating ACT relu / DVE
          2-op form, gathered 8 pairs per [128, 8*320] fp16 tile and
          stored with 2 DMAs per tile (one per row parity). The f16
          wire output is upcast to f32 on the host.

All SBUF pair slots use a 324-element stride with data at offset 2 so
fp16 interiors are 4-byte aligned (enables DVE 2x/4x perf modes).
Weights are pre-transposed on the host (w1t [Cin,3,3,Cout] fp16, w2t
[Cout,3,3,Cout] f32) so weight DMAs have 128-256 B contiguous runs.
Conv biases b1/b2 cancel under InstanceNorm (affine=False) and are
accepted but unused.
"""
import sys
sys.path.insert(0, '/opt/trn_rl_repo')
import contextlib
import numpy as np
import concourse.bacc as bacc_mod
import concourse.tile as tile
import concourse.mybir as mybir
from concourse.ap import AP
from concourse.bass_utils import run_bass_kernel_spmd

f32 = mybir.dt.float32
f32r = mybir.dt.float32r
f16 = mybir.dt.float16
AF = mybir.ActivationFunctionType
OP = mybir.AluOpType

B, CIN, COUT, H, W = 8, 32, 64, 320, 320
WP = W + 2            # matmul rhs window width
WPS = W + 4           # storage stride per pair slot (interior 4B-aligned)
HW = H * W
EPS = 1e-5
NP = H // 2          # 160 conv1 row pairs (h = 0,2,...,318)
NPB = H // 2 - 1     # 159 conv2 row pairs (h = 1,3,...,317)
INV_HW = 1.0 / HW
XG = 8               # conv1 pairs per batched x DMA
COG = 8              # pass-C pairs per batched out DMA
HPOOL = 8            # rotating normalized-h tiles

_CACHE = {}


NG = NP // XG        # 20 batched x-load groups


def _build(repeat=0):
    nc = bacc_mod.Bacc("TRN2", target_bir_lowering=False)
    # host-tiled x: xg[g, j*32+c, s*WP+1+w] = x[c, 2*(8g+s)-1+j, w], 0 padded
    x_d = nc.dram_tensor("xg", [NG, 128, XG * WPS], f16, kind="ExternalInput")
    w1t_d = nc.dram_tensor("w1t", [CIN, 3, 3, COUT], f16, kind="ExternalInput")
    w2t_d = nc.dram_tensor("w2t", [COUT, 3, 3, COUT], f32, kind="ExternalInput")
    out_d = nc.dram_tensor("out", [COUT, H, W], f16, kind="ExternalOutput")

    with tile.TileContext(nc) as tc:
        with contextlib.ExitStack() as ctx:
            wp = ctx.enter_context(tc.tile_pool(name="wp", bufs=1))
            xp = ctx.enter_context(tc.tile_pool(name="xp", bufs=2))
            scr = ctx.enter_context(tc.tile_pool(name="scr", bufs=2))
            cop = ctx.enter_context(tc.tile_pool(name="cop", bufs=3))
            psp = ctx.enter_context(tc.tile_pool(name="psp", bufs=8, space="PSUM"))

            def body(_iv=None):
                zt = wp.tile([128, 128], f32, tag="zt", name="zt", bufs=1)
                nc.gpsimd.memset(zt[:], 0.0)

                bigE = wp.tile([128, NP * WPS], f16, tag="bigE", name="bigE", bufs=1)
                bigE3 = bigE[:].rearrange("p (k w) -> p k w", k=NP)
                # zero every pad column {322k, 322k+321} in one strided memset
                nc.gpsimd.memset(bigE3[:, :, 1:WP + 1:WP - 1], 0.0)

                # ---------------- conv1 weights ---------------------------
                # lhsT[(j,c),(r,o)] = w1[o,c,j-r,kw] = w1t[c, j-r, kw, o]
                # (virtual rows -1/320 are zero in the host-tiled x, so the
                # same "mid" tiles are correct for the edge pairs too)
                lw1 = {}
                for kw in range(3):
                    t = wp.tile([128, 128], f16, tag=f"lw1m{kw}", name=f"lw1m{kw}")
                    nc.vector.tensor_copy(t[:], zt[:])
                    # col half r=0: rows 0:96 <- a=0..2
                    nc.scalar.dma_start(
                        t[0:96, 0:64],
                        AP(w1t_d[:].tensor, kw * 64,
                           [[3 * 64, 3], [9 * 64, 32], [1, 64]]))
                    # col half r=1: rows 32:128 <- a=0..2
                    nc.scalar.dma_start(
                        t[32:128, 64:128],
                        AP(w1t_d[:].tensor, kw * 64,
                           [[3 * 64, 3], [9 * 64, 32], [1, 64]]))
                    lw1[kw] = t

                # ---------------- conv2 weight staging (f32, pre-scale) ---
                # A[(u,i),(r,o)] = w2[o,i,u-r] (u-r in {0,1}) = w2t[i,u-r,kw,o]
                # B[(v,i),(r,o)] = w2[o,i,v+2-r]              = w2t[i,v+2-r,kw,o]
                w2st = {}
                for kw in range(3):
                    sA = wp.tile([128, 128], f32, tag=f"w2sA{kw}", name=f"w2sA{kw}")
                    nc.vector.tensor_copy(sA[0:64, 64:128], zt[0:64, 64:128])
                    # r=0 col half: rows (u)*64, u=0..1 <- kh=u
                    nc.gpsimd.dma_start(
                        sA[:, 0:64],
                        AP(w2t_d[:].tensor, kw * 64,
                           [[3 * 64, 2], [9 * 64, 64], [1, 64]]))
                    # r=1 col half: rows 64:128 (u=1) <- kh=0
                    nc.gpsimd.dma_start(
                        sA[64:128, 64:128],
                        AP(w2t_d[:].tensor, kw * 64, [[9 * 64, 64], [1, 64]]))
                    w2st[("A", kw)] = sA
                    sB = wp.tile([128, 128], f32, tag=f"w2sB{kw}", name=f"w2sB{kw}")
                    nc.vector.tensor_copy(sB[64:128, 0:64], zt[64:128, 0:64])
                    # r=0 col half: rows 0:64 (v=0) <- kh=2
                    nc.gpsimd.dma_start(
                        sB[0:64, 0:64],
                        AP(w2t_d[:].tensor, 2 * 3 * 64 + kw * 64,
                           [[9 * 64, 64], [1, 64]]))
                    # r=1 col half: rows (v)*64, v=0..1 <- kh=v+1
                    nc.gpsimd.dma_start(
                        sB[:, 64:128],
                        AP(w2t_d[:].tensor, 3 * 64 + kw * 64,
                           [[3 * 64, 2], [9 * 64, 64], [1, 64]]))
                    w2st[("B", kw)] = sB
                    # S0: row 0 single (taps kh=1,2 from rows 0,1)
                    s0 = wp.tile([128, 64], f32, tag=f"w2s0{kw}", name=f"w2s0{kw}")
                    nc.gpsimd.dma_start(
                        s0[:, :],
                        AP(w2t_d[:].tensor, 3 * 64 + kw * 64,
                           [[3 * 64, 2], [9 * 64, 64], [1, 64]]))
                    w2st[("S0", kw)] = s0
                    # S9: row H-1 single (taps kh=0,1 from rows 318,319)
                    s9 = wp.tile([128, 64], f32, tag=f"w2s9{kw}", name=f"w2s9{kw}")
                    nc.gpsimd.dma_start(
                        s9[:, :],
                        AP(w2t_d[:].tensor, kw * 64,
                           [[3 * 64, 2], [9 * 64, 64], [1, 64]]))
                    w2st[("S9", kw)] = s9

                # bn_stats records: 6 f32 per (pair, partition)
                st1 = wp.tile([128, NP * 6], f32, tag="st1", name="st1")
                st2 = wp.tile([128, (NPB + 2) * 6], f32, tag="st2", name="st2")

                # ---------------- pass A: conv1 + stats --------------------
                def passA_pair(k, rhs, off):
                    ps = psp.tile([128, W], f32, tag="pp", name=f"psA{k}")
                    for kw in range(3):
                        nc.tensor.matmul(ps[:, :], lw1[kw][:],
                                         rhs[:, off + kw:off + kw + W],
                                         start=(kw == 0), stop=(kw == 2))
                    nc.scalar.activation(bigE3[:, k, 2:W + 2], ps[:], AF.Copy)
                    nc.vector.bn_stats(st1[:, 6 * k:6 * k + 6],
                                       bigE3[:, k, 2:W + 2])

                for gi in range(NG):
                    xg = xp.tile([128, XG * WPS], f16, tag="xg", name=f"xg{gi}")
                    nc.sync.dma_start(xg[:], x_d[gi])
                    for s in range(XG):
                        passA_pair(XG * gi + s, xg, s * WPS + 1)

                # ---------------- stats -> mu, 1/sd ------------------------
                def stats(st, n0, n1, pfx):
                    # st: [128, ncols*6] bn_stats records; partitions 0:64
                    # hold n0 records, 64:128 hold n1 (all records n=320,
                    # bn_aggr's variance combine requires equal counts)
                    agg = wp.tile([128, 2], f32, tag=f"{pfx}agg", name=f"{pfx}agg")
                    nc.vector.bn_aggr(agg[0:64, :], st[0:64, 0:6 * n0])
                    nc.vector.bn_aggr(agg[64:128, :], st[64:128, 0:6 * n1])
                    mean = agg[:, 0:1]
                    var = agg[:, 1:2]
                    ex2 = wp.tile([128, 1], f32, tag=f"{pfx}ex2", name=f"{pfx}ex2")
                    nc.vector.tensor_tensor(ex2[:], mean, mean, OP.mult)
                    nc.vector.tensor_tensor(ex2[:], var, ex2[:], OP.add)
                    fa = wp.tile([64, 1], f32, tag=f"{pfx}fa", name=f"{pfx}fa")
                    fb = wp.tile([64, 1], f32, tag=f"{pfx}fb", name=f"{pfx}fb")
                    nc.sync.dma_start(fa[:], agg[64:128, 0:1])
                    nc.sync.dma_start(fb[:], ex2[64:128, :])
                    w0, w1 = n0 / (n0 + n1), n1 / (n0 + n1)
                    mu = wp.tile([64, 1], f32, tag=f"{pfx}mu", name=f"{pfx}mu")
                    e2 = wp.tile([64, 1], f32, tag=f"{pfx}e2", name=f"{pfx}e2")
                    t = wp.tile([64, 1], f32, tag=f"{pfx}t", name=f"{pfx}t")
                    nc.vector.tensor_scalar(mu[:], mean[0:64], w0, None, OP.mult)
                    nc.vector.tensor_scalar(t[:], fa[:], w1, None, OP.mult)
                    nc.vector.tensor_tensor(mu[:], mu[:], t[:], OP.add)
                    nc.vector.tensor_scalar(e2[:], ex2[0:64, :], w0, None, OP.mult)
                    nc.vector.tensor_scalar(t[:], fb[:], w1, None, OP.mult)
                    nc.vector.tensor_tensor(e2[:], e2[:], t[:], OP.add)
                    varo = wp.tile([64, 1], f32, tag=f"{pfx}varo", name=f"{pfx}varo")
                    nc.vector.tensor_tensor(varo[:], mu[:], mu[:], OP.mult)
                    nc.vector.tensor_tensor(varo[:], e2[:], varo[:], OP.subtract)
                    nc.vector.tensor_scalar(varo[:], varo[:], EPS, None, OP.add)
                    sd = wp.tile([64, 1], f32, tag=f"{pfx}sd", name=f"{pfx}sd")
                    nc.scalar.activation(sd[:], varo[:], AF.Sqrt)
                    s = wp.tile([64, 1], f32, tag=f"{pfx}s", name=f"{pfx}s")
                    nc.vector.reciprocal(s[:], sd[:])
                    return mu, s

                def bcast128(src64, tag):
                    t = wp.tile([128, 1], f32, tag=tag, name=tag)
                    nc.sync.dma_start(t[0:64, :], src64[:])
                    nc.sync.dma_start(t[64:128, :], src64[:])
                    return t

                mu1, s1 = stats(st1, NP, NP, "st1")
                negmu1 = wp.tile([64, 1], f32, tag="negmu1", name="negmu1")
                nc.vector.tensor_scalar(negmu1[:], mu1[:], -1.0, None, OP.mult)
                nmu1v = bcast128(negmu1, "nmu1v")
                s1v = bcast128(s1, "s1v")

                # scale staged conv2 weights by s1 (per input channel = partition)
                lw2 = {}
                for key, st in w2st.items():
                    cols = st.shape[-1]
                    t = wp.tile([128, cols], f16, tag=f"lw2{key[0]}{key[1]}",
                                name=f"lw2{key[0]}{key[1]}")
                    nc.vector.tensor_scalar(t[:], st[:], s1v[:, 0:1], None, OP.mult)
                    lw2[key] = t

                # ---------------- pass B: conv2 + stats --------------------
                # slot 0 is pinned to h[0] (the row-0 single consumes it after
                # the pair loop); slots 1..HPOOL rotate for i >= 1
                bigH = wp.tile([128, (HPOOL + 1) * WPS], f16, tag="bigH",
                               name="bigH", bufs=1)
                bigH3 = bigH[:].rearrange("p (k w) -> p k w", k=HPOOL + 1)
                nc.gpsimd.memset(bigH3[:, :, 1:WP + 1:WP - 1], 0.0)
                hk = {}

                def norm(i):
                    # h[i] = relu(e[i] - mu1), fp16, rotating slot
                    sl = 0 if i == 0 else 1 + (i - 1) % HPOOL
                    nc.vector.tensor_scalar(bigH3[:, sl, 2:W + 2],
                                            bigE3[:, i, 2:W + 2],
                                            nmu1v[:, 0:1], 0.0, OP.add, OP.max)
                    hk[i] = bigH[:, sl * WPS + 1:sl * WPS + 1 + WP]

                norm(0)
                norm(1)
                norm(2)
                h0 = hk[0]   # slot 0 is pinned; row-0 single runs post-loop

                for kb in range(NPB):
                    if kb + 3 <= NP - 1:
                        norm(kb + 3)
                    eA, eB = hk.pop(kb), hk[kb + 1]
                    ps = psp.tile([128, W], f32, tag="pp", name=f"psB{kb}")
                    for kw in range(3):
                        nc.tensor.matmul(ps[:, :], lw2[("A", kw)][:],
                                         eA[:, kw:kw + W],
                                         start=(kw == 0), stop=False)
                    for kw in range(3):
                        nc.tensor.matmul(ps[:, :], lw2[("B", kw)][:],
                                         eB[:, kw:kw + W],
                                         start=False, stop=(kw == 2))
                    # y2 pair kb lands bf16 over the e slot it just retired
                    nc.scalar.activation(bigE3[:, kb, 2:W + 2], ps[:], AF.Copy)
                    nc.vector.bn_stats(st2[:, 6 * kb:6 * kb + 6],
                                       bigE3[:, kb, 2:W + 2])

                # single row 0: taps kh=1,2 from rows 0,1 (h[0])
                ps0 = psp.tile([64, W], f32, tag="pp", name="psS0")
                for kw in range(3):
                    nc.tensor.matmul(ps0[:, :], lw2[("S0", kw)][:],
                                     h0[:, kw:kw + W],
                                     start=(kw == 0), stop=(kw == 2))
                y0 = wp.tile([64, W], f32, tag="ys0", name="ys0")
                nc.scalar.activation(y0[:], ps0[:], AF.Copy)
                nc.vector.bn_stats(st2[0:64, 6 * NPB:6 * NPB + 6], y0[:])

                # single row 319: taps kh=0,1 from rows 318,319 (h[159])
                e9 = hk[NP - 1]
                ps9 = psp.tile([64, W], f32, tag="pp", name="psS9")
                for kw in range(3):
                    nc.tensor.matmul(ps9[:, :], lw2[("S9", kw)][:],
                                     e9[:, kw:kw + W],
                                     start=(kw == 0), stop=(kw == 2))
                y9 = wp.tile([64, W], f32, tag="ys9", name="ys9")
                nc.scalar.activation(y9[:], ps9[:], AF.Copy)
                nc.vector.bn_stats(st2[0:64, 6 * (NPB + 1):6 * (NPB + 2)], y9[:])

                # ---------------- stats2 -> s2, t2 = -mu2*s2 ---------------
                mu2, s2 = stats(st2, NPB + 2, NPB, "st2")
                t2 = wp.tile([64, 1], f32, tag="t2", name="t2")
                nc.vector.tensor_tensor(t2[:], mu2[:], s2[:], OP.mult)
                nc.vector.tensor_scalar(t2[:], t2[:], -1.0, None, OP.mult)
                s2v = bcast128(s2, "s2v")
                t2v = bcast128(t2, "t2v")
                negmu2 = wp.tile([64, 1], f32, tag="negmu2", name="negmu2")
                nc.vector.tensor_scalar(negmu2[:], mu2[:], -1.0, None, OP.mult)
                nmu2v = bcast128(negmu2, "nmu2v")

                # ---------------- pass C: out = relu(y2*s2 + t2) -----------
                co0 = wp.tile([64, W], f16, tag="co0", name="co0")
                nc.scalar.activation(co0[:], y0[:], AF.Relu,
                                     bias=t2v[0:64, 0:1], scale=s2v[0:64, 0:1])
                nc.sync.dma_start(out_d[:, 0, :], co0[:])

                cgroups = []
                kb0 = 0
                while kb0 < NPB:
                    g = min(COG, NPB - kb0)
                    cgroups.append((kb0, g))
                    kb0 += g
                for ci, (kb0, g) in enumerate(cgroups):
                    co = cop.tile([128, COG * W], f16, tag="co", name=f"co{ci}")
                    for q in range(g):
                        kb = kb0 + q
                        dst = co[:, q * W:(q + 1) * W]
                        ysrc = bigE3[:, kb, 2:W + 2]
                        if kb % 5 >= 3:
                            nc.scalar.activation(dst, ysrc, AF.Relu,
                                                 bias=t2v[:, 0:1],
                                                 scale=s2v[:, 0:1])
                        else:
                            # relu((y-mu2)*s2) = max((y + -mu2)*s2, 0), s2>0
                            nc.vector.tensor_scalar(dst, ysrc, nmu2v[:, 0:1],
                                                    None, OP.add)
                            nc.vector.tensor_scalar(dst, dst, s2v[:, 0:1], 0.0,
                                                    OP.mult, OP.max)
                    # dst[c,(q,w)] = out[c, 2*(kb0+q)+1+r, w], one DMA per r
                    co3 = co[:].rearrange("p (q w) -> p q w", w=W)
                    for r in range(2):
                        eng = nc.sync if r == 0 else nc.gpsimd
                        eng.dma_start(
                            AP(out_d[:].tensor, (2 * kb0 + 1 + r) * W,
                               [[HW, COUT], [2 * W, g], [1, W]]),
                            co3[r * 64:(r + 1) * 64, 0:g, :])

                co9 = wp.tile([64, W], f16, tag="co9", name="co9")
                nc.scalar.activation(co9[:], y9[:], AF.Relu,
                                     bias=t2v[0:64, 0:1], scale=s2v[0:64, 0:1])
                nc.sync.dma_start(out_d[:, H - 1, :], co9[:])

            if repeat:
                with tc.For_i(0, repeat, 1, hint_engines=(mybir.EngineType.PE,)):
                    body()
            else:
                body()

    nc.finalize()
    return nc


def _get_nc(repeat=0):
    key = ("nc", repeat)
    if key not in _CACHE:
        _CACHE[key] = _build(repeat)
    return _CACHE[key]


def _tile_x(xi):
    # xg[g, j*32+c, s*WPS+2+w] = x[c, 2*(8g+s)-1+j, w], zero padded, fp16
    # (data starts at slot offset 2 so fp16 interiors are 4B-aligned)
    xpad = np.zeros((CIN, H + 2, W), np.float16)
    xpad[:, 1:H + 1] = xi
    rows = 2 * np.arange(NP)[:, None] + np.arange(4)[None, :]   # [NP,4]
    xt = np.zeros((NP, 4, CIN, WPS), np.float16)
    xt[..., 2:W + 2] = xpad[:, rows, :].transpose(1, 2, 0, 3)
    return np.ascontiguousarray(
        xt.reshape(NG, XG, 128, WPS).transpose(0, 2, 1, 3).reshape(NG, 128, XG * WPS))


def _in_map(xi, w1, w2):
    w1t = np.ascontiguousarray(w1.transpose(1, 2, 3, 0).astype(np.float16))
    w2t = np.ascontiguousarray(w2.transpose(1, 2, 3, 0))
    return {"xg": _tile_x(np.asarray(xi, np.float16)), "w1t": w1t, "w2t": w2t}


def kernel(x, w1, b1=None, w2=None, b2=None, **kw):
    x = np.ascontiguousarray(np.asarray(x, dtype=np.float32))
    w1 = np.ascontiguousarray(np.asarray(w1, dtype=np.float32))
    w2 = np.ascontiguousarray(np.asarray(w2, dtype=np.float32))
    nc = _get_nc()
    in_maps = [_in_map(x[i], w1, w2) for i in range(B)]
    res = run_bass_kernel_spmd(nc, in_maps, list(range(B)), trace=False)
    return np.stack([res.results[i]["out"].astype(np.float32) for i in range(B)],
                    axis=0)


# revision 4
# speedup vs baseline: 1.0883x; 1.0883x over previous
"""Fused Conv3x3-InstanceNorm-ReLU x2 block for Trainium2 (fp16 path).

Data-parallel over 8 NeuronCores (one batch sample per core). Per-core:

  pass A: conv1 as row-pair matmuls (fp16, K=128 = 4 row-shifted Cin
          blocks, M=128 = 2 out rows x 64 Cout, N=320, fp32 PSUM).
          x is host-tiled into 20 contiguous [128, 8*324] fp16 groups
          (pads and edge rows pre-zeroed) so pass A needs exactly 20
          big DMAs. Pairs are processed in couples sharing a 2-bank
          PSUM tile: one strided ACT op evacuates both pairs into a
          big fp16 SBUF arena (amortizing ACT's fixed op cost); DVE
          bn_stats records (count, mean, M2) per pair and partition.
  norm1:  h = relu(y1 - mu1) (DVE, fp16 4x mode); the rsqrt scale s1
          is folded into the conv2 weights per input channel.
  pass B: conv2 on normalized pairs (2 K-blocks of 64 ch, 6 matmuls).
          y2 lands fp16 IN PLACE over the arena slot its eA input just
          retired -- zero spill, zero extra SBUF. Row 0/319 singles run
          after the pair loop so PE enters pass B sooner.
  stats:  per-half bn_aggr (equal-count records per half) + count-
          weighted combine across the two row-parity halves.
  pass C: out = relu((y2 - mu2) * s2), alternating ACT relu / DVE
          2-op form, gathered 8 pairs per [128, 8*320] fp16 tile and
          stored with 2 DMAs per tile (one per row parity). The f16
          wire output is upcast to f32 on the host.

All SBUF pair slots use a 324-element stride with data at offset 2 so
fp16 interiors are 4-byte aligned (enables DVE 2x/4x perf modes).
Weights are pre-transposed on the host (w1t [Cin,3,3,Cout] fp16, w2t
[Cout,3,3,Cout] f32) so weight DMAs have 128-256 B contiguous runs.
Conv biases b1/b2 cancel under InstanceNorm (affine=False) and are
accepted but unused.
"""
import sys
sys.path.insert(0, '/opt/trn_rl_repo')
import contextlib
import numpy as np
import concourse.bacc as bacc_mod
import concourse.tile as tile
import concourse.mybir as mybir
from concourse.ap import AP
from concourse.bass_utils import run_bass_kernel_spmd

f32 = mybir.dt.float32
f32r = mybir.dt.float32r
f16 = mybir.dt.float16
AF = mybir.ActivationFunctionType
OP = mybir.AluOpType

B, CIN, COUT, H, W = 8, 32, 64, 320, 320
WP = W + 2            # matmul rhs window width
WPS = W + 4           # storage stride per pair slot (interior 4B-aligned)
HW = H * W
EPS = 1e-5
NP = H // 2          # 160 conv1 row pairs (h = 0,2,...,318)
NPB = H // 2 - 1     # 159 conv2 row pairs (h = 1,3,...,317)
INV_HW = 1.0 / HW
XG = 8               # conv1 pairs per batched x DMA
COG = 8              # pass-C pairs per batched out DMA
HPOOL = 8            # rotating normalized-h tiles

_CACHE = {}


NG = NP // XG        # 20 batched x-load groups


def _build(repeat=0):
    nc = bacc_mod.Bacc("TRN2", target_bir_lowering=False)
    # host-tiled x: xg[g, j*32+c, s*WP+1+w] = x[c, 2*(8g+s)-1+j, w], 0 padded
    x_d = nc.dram_tensor("xg", [NG, 128, XG * WPS], f16, kind="ExternalInput")
    w1t_d = nc.dram_tensor("w1t", [CIN, 3, 3, COUT], f16, kind="ExternalInput")
    w2t_d = nc.dram_tensor("w2t", [COUT, 3, 3, COUT], f32, kind="ExternalInput")
    out_d = nc.dram_tensor("out", [COUT, H, W], f16, kind="ExternalOutput")

    with tile.TileContext(nc) as tc:
        with contextlib.ExitStack() as ctx:
            wp = ctx.enter_context(tc.tile_pool(name="wp", bufs=1))
            xp = ctx.enter_context(tc.tile_pool(name="xp", bufs=2))
            scr = ctx.enter_context(tc.tile_pool(name="scr", bufs=2))
            cop = ctx.enter_context(tc.tile_pool(name="cop", bufs=3))
            psp = ctx.enter_context(tc.tile_pool(name="psp", bufs=4, space="PSUM"))

            def body(_iv=None):
                zt = wp.tile([128, 128], f32, tag="zt", name="zt", bufs=1)
                nc.gpsimd.memset(zt[:], 0.0)

                bigE = wp.tile([128, NP * WPS], f16, tag="bigE", name="bigE", bufs=1)
                bigE3 = bigE[:].rearrange("p (k w) -> p k w", k=NP)
                # zero every pad column {322k, 322k+321} in one strided memset
                nc.gpsimd.memset(bigE3[:, :, 1:WP + 1:WP - 1], 0.0)

                # ---------------- conv1 weights ---------------------------
                # lhsT[(j,c),(r,o)] = w1[o,c,j-r,kw] = w1t[c, j-r, kw, o]
                # (virtual rows -1/320 are zero in the host-tiled x, so the
                # same "mid" tiles are correct for the edge pairs too)
                lw1 = {}
                for kw in range(3):
                    t = wp.tile([128, 128], f16, tag=f"lw1m{kw}", name=f"lw1m{kw}")
                    nc.vector.tensor_copy(t[:], zt[:])
                    # col half r=0: rows 0:96 <- a=0..2
                    nc.scalar.dma_start(
                        t[0:96, 0:64],
                        AP(w1t_d[:].tensor, kw * 64,
                           [[3 * 64, 3], [9 * 64, 32], [1, 64]]))
                    # col half r=1: rows 32:128 <- a=0..2
                    nc.scalar.dma_start(
                        t[32:128, 64:128],
                        AP(w1t_d[:].tensor, kw * 64,
                           [[3 * 64, 3], [9 * 64, 32], [1, 64]]))
                    lw1[kw] = t

                # ---------------- conv2 weight staging (f32, pre-scale) ---
                # A[(u,i),(r,o)] = w2[o,i,u-r] (u-r in {0,1}) = w2t[i,u-r,kw,o]
                # B[(v,i),(r,o)] = w2[o,i,v+2-r]              = w2t[i,v+2-r,kw,o]
                w2st = {}
                for kw in range(3):
                    sA = wp.tile([128, 128], f32, tag=f"w2sA{kw}", name=f"w2sA{kw}")
                    nc.vector.tensor_copy(sA[0:64, 64:128], zt[0:64, 64:128])
                    # r=0 col half: rows (u)*64, u=0..1 <- kh=u
                    nc.gpsimd.dma_start(
                        sA[:, 0:64],
                        AP(w2t_d[:].tensor, kw * 64,
                           [[3 * 64, 2], [9 * 64, 64], [1, 64]]))
                    # r=1 col half: rows 64:128 (u=1) <- kh=0
                    nc.gpsimd.dma_start(
                        sA[64:128, 64:128],
                        AP(w2t_d[:].tensor, kw * 64, [[9 * 64, 64], [1, 64]]))
                    w2st[("A", kw)] = sA
                    sB = wp.tile([128, 128], f32, tag=f"w2sB{kw}", name=f"w2sB{kw}")
                    nc.vector.tensor_copy(sB[64:128, 0:64], zt[64:128, 0:64])
                    # r=0 col half: rows 0:64 (v=0) <- kh=2
                    nc.gpsimd.dma_start(
                        sB[0:64, 0:64],
                        AP(w2t_d[:].tensor, 2 * 3 * 64 + kw * 64,
                           [[9 * 64, 64], [1, 64]]))
                    # r=1 col half: rows (v)*64, v=0..1 <- kh=v+1
                    nc.gpsimd.dma_start(
                        sB[:, 64:128],
                        AP(w2t_d[:].tensor, 3 * 64 + kw * 64,
                           [[3 * 64, 2], [9 * 64, 64], [1, 64]]))
                    w2st[("B", kw)] = sB
                    # S0: row 0 single (taps kh=1,2 from rows 0,1)
                    s0 = wp.tile([128, 64], f32, tag=f"w2s0{kw}", name=f"w2s0{kw}")
                    nc.gpsimd.dma_start(
                        s0[:, :],
                        AP(w2t_d[:].tensor, 3 * 64 + kw * 64,
                           [[3 * 64, 2], [9 * 64, 64], [1, 64]]))
                    w2st[("S0", kw)] = s0
                    # S9: row H-1 single (taps kh=0,1 from rows 318,319)
                    s9 = wp.tile([128, 64], f32, tag=f"w2s9{kw}", name=f"w2s9{kw}")
                    nc.gpsimd.dma_start(
                        s9[:, :],
                        AP(w2t_d[:].tensor, kw * 64,
                           [[3 * 64, 2], [9 * 64, 64], [1, 64]]))
                    w2st[("S9", kw)] = s9

                # bn_stats records: 6 f32 per (pair, partition)
                st1 = wp.tile([128, NP * 6], f32, tag="st1", name="st1")
                st2 = wp.tile([128, (NPB + 2) * 6], f32, tag="st2", name="st2")

                # ---------------- pass A: conv1 + stats --------------------
                # couples: 2 pairs share a [128,1024] 2-bank psum tile and
                # one strided ACT evac (amortizes ACT's fixed op cost)
                def passA_couple(k0, rhs):
                    ps2 = psp.tile([128, 1024], f32, tag="pp", name=f"psA{k0}")
                    for h2 in range(2):
                        off = (k0 % XG + h2) * WPS + 1
                        for kw in range(3):
                            nc.tensor.matmul(ps2[:, 512 * h2:512 * h2 + W],
                                             lw1[kw][:],
                                             rhs[:, off + kw:off + kw + W],
                                             start=(kw == 0), stop=(kw == 2))
                    ev = ps2[:].rearrange("p (b w) -> p b w", b=2)[:, :, 0:W]
                    nc.scalar.activation(bigE3[:, k0:k0 + 2, 2:W + 2], ev,
                                         AF.Copy)
                    for k in (k0, k0 + 1):
                        nc.vector.bn_stats(st1[:, 6 * k:6 * k + 6],
                                           bigE3[:, k, 2:W + 2])

                for gi in range(NG):
                    xg = xp.tile([128, XG * WPS], f16, tag="xg", name=f"xg{gi}")
                    nc.sync.dma_start(xg[:], x_d[gi])
                    for s2 in range(0, XG, 2):
                        passA_couple(XG * gi + s2, xg)

                # ---------------- stats -> mu, 1/sd ------------------------
                def stats(st, n0, n1, pfx):
                    # st: [128, ncols*6] bn_stats records; partitions 0:64
                    # hold n0 records, 64:128 hold n1 (all records n=320,
                    # bn_aggr's variance combine requires equal counts)
                    agg = wp.tile([128, 2], f32, tag=f"{pfx}agg", name=f"{pfx}agg")
                    nc.vector.bn_aggr(agg[0:64, :], st[0:64, 0:6 * n0])
                    nc.vector.bn_aggr(agg[64:128, :], st[64:128, 0:6 * n1])
                    mean = agg[:, 0:1]
                    var = agg[:, 1:2]
                    ex2 = wp.tile([128, 1], f32, tag=f"{pfx}ex2", name=f"{pfx}ex2")
                    nc.vector.tensor_tensor(ex2[:], mean, mean, OP.mult)
                    nc.vector.tensor_tensor(ex2[:], var, ex2[:], OP.add)
                    fa = wp.tile([64, 1], f32, tag=f"{pfx}fa", name=f"{pfx}fa")
                    fb = wp.tile([64, 1], f32, tag=f"{pfx}fb", name=f"{pfx}fb")
                    nc.sync.dma_start(fa[:], agg[64:128, 0:1])
                    nc.sync.dma_start(fb[:], ex2[64:128, :])
                    w0, w1 = n0 / (n0 + n1), n1 / (n0 + n1)
                    mu = wp.tile([64, 1], f32, tag=f"{pfx}mu", name=f"{pfx}mu")
                    e2 = wp.tile([64, 1], f32, tag=f"{pfx}e2", name=f"{pfx}e2")
                    t = wp.tile([64, 1], f32, tag=f"{pfx}t", name=f"{pfx}t")
                    nc.vector.tensor_scalar(mu[:], mean[0:64], w0, None, OP.mult)
                    nc.vector.tensor_scalar(t[:], fa[:], w1, None, OP.mult)
                    nc.vector.tensor_tensor(mu[:], mu[:], t[:], OP.add)
                    nc.vector.tensor_scalar(e2[:], ex2[0:64, :], w0, None, OP.mult)
                    nc.vector.tensor_scalar(t[:], fb[:], w1, None, OP.mult)
                    nc.vector.tensor_tensor(e2[:], e2[:], t[:], OP.add)
                    varo = wp.tile([64, 1], f32, tag=f"{pfx}varo", name=f"{pfx}varo")
                    nc.vector.tensor_tensor(varo[:], mu[:], mu[:], OP.mult)
                    nc.vector.tensor_tensor(varo[:], e2[:], varo[:], OP.subtract)
                    nc.vector.tensor_scalar(varo[:], varo[:], EPS, None, OP.add)
                    sd = wp.tile([64, 1], f32, tag=f"{pfx}sd", name=f"{pfx}sd")
                    nc.scalar.activation(sd[:], varo[:], AF.Sqrt)
                    s = wp.tile([64, 1], f32, tag=f"{pfx}s", name=f"{pfx}s")
                    nc.vector.reciprocal(s[:], sd[:])
                    return mu, s

                def bcast128(src64, tag):
                    t = wp.tile([128, 1], f32, tag=tag, name=tag)
                    nc.sync.dma_start(t[0:64, :], src64[:])
                    nc.sync.dma_start(t[64:128, :], src64[:])
                    return t

                mu1, s1 = stats(st1, NP, NP, "st1")
                negmu1 = wp.tile([64, 1], f32, tag="negmu1", name="negmu1")
                nc.vector.tensor_scalar(negmu1[:], mu1[:], -1.0, None, OP.mult)
                nmu1v = bcast128(negmu1, "nmu1v")
                s1v = bcast128(s1, "s1v")

                # scale staged conv2 weights by s1 (per input channel = partition)
                lw2 = {}
                for key, st in w2st.items():
                    cols = st.shape[-1]
                    t = wp.tile([128, cols], f16, tag=f"lw2{key[0]}{key[1]}",
                                name=f"lw2{key[0]}{key[1]}")
                    nc.vector.tensor_scalar(t[:], st[:], s1v[:, 0:1], None, OP.mult)
                    lw2[key] = t

                # ---------------- pass B: conv2 + stats --------------------
                # slot 0 is pinned to h[0] (the row-0 single consumes it after
                # the pair loop); slots 1..HPOOL rotate for i >= 1
                bigH = wp.tile([128, (HPOOL + 1) * WPS], f16, tag="bigH",
                               name="bigH", bufs=1)
                bigH3 = bigH[:].rearrange("p (k w) -> p k w", k=HPOOL + 1)
                nc.gpsimd.memset(bigH3[:, :, 1:WP + 1:WP - 1], 0.0)
                hk = {}

                def norm(i):
                    # h[i] = relu(e[i] - mu1), fp16, rotating slot
                    sl = 0 if i == 0 else 1 + (i - 1) % HPOOL
                    nc.vector.tensor_scalar(bigH3[:, sl, 2:W + 2],
                                            bigE3[:, i, 2:W + 2],
                                            nmu1v[:, 0:1], 0.0, OP.add, OP.max)
                    hk[i] = bigH[:, sl * WPS + 1:sl * WPS + 1 + WP]

                norm(0)
                norm(1)
                norm(2)
                h0 = hk[0]   # slot 0 is pinned; row-0 single runs post-loop

                for kb in range(NPB):
                    if kb + 3 <= NP - 1:
                        norm(kb + 3)
                    eA, eB = hk.pop(kb), hk[kb + 1]
                    ps = psp.tile([128, 1024], f32, tag="pp",
                                  name=f"psB{kb}")[:, 0:W]
                    for kw in range(3):
                        nc.tensor.matmul(ps[:, :], lw2[("A", kw)][:],
                                         eA[:, kw:kw + W],
                                         start=(kw == 0), stop=False)
                    for kw in range(3):
                        nc.tensor.matmul(ps[:, :], lw2[("B", kw)][:],
                                         eB[:, kw:kw + W],
                                         start=False, stop=(kw == 2))
                    # y2 pair kb lands bf16 over the e slot it just retired
                    nc.scalar.activation(bigE3[:, kb, 2:W + 2], ps[:], AF.Copy)
                    nc.vector.bn_stats(st2[:, 6 * kb:6 * kb + 6],
                                       bigE3[:, kb, 2:W + 2])

                # single row 0: taps kh=1,2 from rows 0,1 (h[0])
                ps0 = psp.tile([128, 1024], f32, tag="pp",
                               name="psS0")[0:64, 0:W]
                for kw in range(3):
                    nc.tensor.matmul(ps0[:, :], lw2[("S0", kw)][:],
                                     h0[:, kw:kw + W],
                                     start=(kw == 0), stop=(kw == 2))
                y0 = wp.tile([64, W], f32, tag="ys0", name="ys0")
                nc.scalar.activation(y0[:], ps0[:], AF.Copy)
                nc.vector.bn_stats(st2[0:64, 6 * NPB:6 * NPB + 6], y0[:])

                # single row 319: taps kh=0,1 from rows 318,319 (h[159])
                e9 = hk[NP - 1]
                ps9 = psp.tile([128, 1024], f32, tag="pp",
                               name="psS9")[0:64, 0:W]
                for kw in range(3):
                    nc.tensor.matmul(ps9[:, :], lw2[("S9", kw)][:],
                                     e9[:, kw:kw + W],
                                     start=(kw == 0), stop=(kw == 2))
                y9 = wp.tile([64, W], f32, tag="ys9", name="ys9")
                nc.scalar.activation(y9[:], ps9[:], AF.Copy)
                nc.vector.bn_stats(st2[0:64, 6 * (NPB + 1):6 * (NPB + 2)], y9[:])

                # ---------------- stats2 -> s2, t2 = -mu2*s2 ---------------
                mu2, s2 = stats(st2, NPB + 2, NPB, "st2")
                t2 = wp.tile([64, 1], f32, tag="t2", name="t2")
                nc.vector.tensor_tensor(t2[:], mu2[:], s2[:], OP.mult)
                nc.vector.tensor_scalar(t2[:], t2[:], -1.0, None, OP.mult)
                s2v = bcast128(s2, "s2v")
                t2v = bcast128(t2, "t2v")
                negmu2 = wp.tile([64, 1], f32, tag="negmu2", name="negmu2")
                nc.vector.tensor_scalar(negmu2[:], mu2[:], -1.0, None, OP.mult)
                nmu2v = bcast128(negmu2, "nmu2v")

                # ---------------- pass C: out = relu(y2*s2 + t2) -----------
                co0 = wp.tile([64, W], f16, tag="co0", name="co0")
                nc.scalar.activation(co0[:], y0[:], AF.Relu,
                                     bias=t2v[0:64, 0:1], scale=s2v[0:64, 0:1])
                nc.sync.dma_start(out_d[:, 0, :], co0[:])

                cgroups = []
                kb0 = 0
                while kb0 < NPB:
                    g = min(COG, NPB - kb0)
                    cgroups.append((kb0, g))
                    kb0 += g
                for ci, (kb0, g) in enumerate(cgroups):
                    co = cop.tile([128, COG * W], f16, tag="co", name=f"co{ci}")
                    for q in range(g):
                        kb = kb0 + q
                        dst = co[:, q * W:(q + 1) * W]
                        ysrc = bigE3[:, kb, 2:W + 2]
                        if kb % 5 >= 3:
                            nc.scalar.activation(dst, ysrc, AF.Relu,
                                                 bias=t2v[:, 0:1],
                                                 scale=s2v[:, 0:1])
                        else:
                            # relu((y-mu2)*s2) = max((y + -mu2)*s2, 0), s2>0
                            nc.vector.tensor_scalar(dst, ysrc, nmu2v[:, 0:1],
                                                    None, OP.add)
                            nc.vector.tensor_scalar(dst, dst, s2v[:, 0:1], 0.0,
                                                    OP.mult, OP.max)
                    # dst[c,(q,w)] = out[c, 2*(kb0+q)+1+r, w], one DMA per r
                    co3 = co[:].rearrange("p (q w) -> p q w", w=W)
                    for r in range(2):
                        eng = nc.sync if r == 0 else nc.gpsimd
                        eng.dma_start(
                            AP(out_d[:].tensor, (2 * kb0 + 1 + r) * W,
                               [[HW, COUT], [2 * W, g], [1, W]]),
                            co3[r * 64:(r + 1) * 64, 0:g, :])

                co9 = wp.tile([64, W], f16, tag="co9", name="co9")
                nc.scalar.activation(co9[:], y9[:], AF.Relu,
                                     bias=t2v[0:64, 0:1], scale=s2v[0:64, 0:1])
                nc.sync.dma_start(out_d[:, H - 1, :], co9[:])

            if repeat:
                with tc.For_i(0, repeat, 1, hint_engines=(mybir.EngineType.PE,)):
                    body()
            else:
                body()

    nc.finalize()
    return nc


def _get_nc(repeat=0):
    key = ("nc", repeat)
    if key not in _CACHE:
        _CACHE[key] = _build(repeat)
    return _CACHE[key]


def _tile_x(xi):
    # xg[g, j*32+c, s*WPS+2+w] = x[c, 2*(8g+s)-1+j, w], zero padded, fp16
    # (data starts at slot offset 2 so fp16 interiors are 4B-aligned)
    xpad = np.zeros((CIN, H + 2, W), np.float16)
    xpad[:, 1:H + 1] = xi
    rows = 2 * np.arange(NP)[:, None] + np.arange(4)[None, :]   # [NP,4]
    xt = np.zeros((NP, 4, CIN, WPS), np.float16)
    xt[..., 2:W + 2] = xpad[:, rows, :].transpose(1, 2, 0, 3)
    return np.ascontiguousarray(
        xt.reshape(NG, XG, 128, WPS).transpose(0, 2, 1, 3).reshape(NG, 128, XG * WPS))


def _in_map(xi, w1, w2):
    w1t = np.ascontiguousarray(w1.transpose(1, 2, 3, 0).astype(np.float16))
    w2t = np.ascontiguousarray(w2.transpose(1, 2, 3, 0))
    return {"xg": _tile_x(np.asarray(xi, np.float16)), "w1t": w1t, "w2t": w2t}


def kernel(x, w1, b1=None, w2=None, b2=None, **kw):
    x = np.ascontiguousarray(np.asarray(x, dtype=np.float32))
    w1 = np.ascontiguousarray(np.asarray(w1, dtype=np.float32))
    w2 = np.ascontiguousarray(np.asarray(w2, dtype=np.float32))
    nc = _get_nc()
    in_maps = [_in_map(x[i], w1, w2) for i in range(B)]
    res = run_bass_kernel_spmd(nc, in_maps, list(range(B)), trace=False)
    return np.stack([res.results[i]["out"].astype(np.float32) for i in range(B)],
                    axis=0)
